# revision 9
# baseline (speedup 1.0000x reference)
"""GNN mean-aggregator (h = xW^T + b; out[i] = mean_{(i,j) in E} h[j]) on 8 trn2 cores.

Strategy (graph/data parallel over destination nodes):
  - Each core owns a contiguous range of 6250 destination nodes.
  - Phase 1 (on device): project x -> h = xW^T (+b) in fp16 (64 features),
    stored to a DRAM scratch in partition-major layout: h row of node n
    lives at flat 128B-row index (n%128)*392 + n//128.  Phase-1 stores are
    wide and contiguous (full DMA rate).
  - Phase 2: per-edge dma_gather of 128-byte h rows (elem_size=64 fp16,
    elem_step=128 -> 256B source stride, i.e. rows are fetched in pairs'
    address space; an even/odd split of edges by source chunk parity keeps
    gather indices within int16 and the 256B-stride requirement).  This
    halves per-edge DMA bytes vs gathering 256B x rows.  TensorE matmuls
    with the one-hot (dest-local) matrix stationary and the gathered
    64-feature rows moving accumulate [dest, feat] blocks in PSUM.
    Scale by 1/deg on DVE and store (partition-major, un-permuted on host).
  - Host sorts edges by destination block and source parity, pads each
    (block, parity) edge list to whole 128-edge chunks uniformly across
    cores (SPMD: one program, per-core data).
"""
import sys

sys.path.insert(0, "/opt/trn_rl_repo")

from contextlib import ExitStack

import numpy as np

from concourse import bass, bacc, mybir, tile
from concourse.bass_utils import run_bass_kernel_spmd

N_NODES = 50000
N_EDGES = 800000
D_IN = 128
D_OUT = 64
N_CORES = 8
NPC = N_NODES // N_CORES      # 6250 destination nodes per core
P = 128
NBLK = (NPC + P - 1) // P     # 49 blocks of 128 destinations
NPAD = NBLK * P               # 6272 padded destinations
NCH = 392                     # x chunks (padded): 392*128 = 50176 >= 50000
NXP = NCH * P                 # padded node count
NPAIR = (NCH // 2) * P        # 25088 gatherable row-pairs (int16-safe)
SBN = 8                       # blocks per superblock
G_SLAB = 16                   # x chunks per phase-1 slab load
G_H = 8                       # chunks per phase-1 psum/store group
NQ = 4                        # SWDGE queues

_prog_cache = {}
last_results = None  # test harness introspection


def _gather128(nc, out_ap, in_ap, idxs_ap, num_idxs, queue_num):
    """dma_gather of 128B rows from a 256B-stride table (elem_size=64 fp16,
    elem_step=128).  Replicates bass.dma_gather minus its elem_size%256
    assert, which the Q7 ucode only requires for transpose mode."""
    eng = nc.gpsimd
    elem_size, elem_step = D_OUT, 2 * D_OUT
    stride_bytes_256 = (elem_step * 2) // 256
    _in_ap = eng.lower_ap_dma(in_ap, for_custom_bir_dma=True)
    _idxs_ap = eng.lower_ap(idxs_ap)
    _out_ap = eng.lower_ap(out_ap)
    return eng.add_instruction(
        mybir.InstDMAGatherAnt(
            name=eng.bass.get_next_instruction_name(),
            ins=[
                *_in_ap,
                _idxs_ap,
                eng.lower_val_access(eng.to_reg(num_idxs)),
            ],
            outs=[_out_ap],
            transpose=False,
            num_idxs=num_idxs,
            elem_size=elem_size,
            stride_bytes_256=stride_bytes_256,
            gen_mode=0,
            single_packet=False,
            queue_num=queue_num,
            sbuf_tokens_per_rank=0,
            sbuf_free_dim_per_rank=0,
            sbuf_free_dim_pad_per_rank=0,
            sbuf_byte_offset=0,
        )
    )


def _build_program(CA, CB, has_bias):
    """CA/CB: per-block chunk counts for even/odd source parity (uniform
    across cores)."""
    CA = list(CA)
    CB = list(CB)
    Ctot = sum(CA) + sum(CB)
    f16 = mybir.dt.float16
    f32 = mybir.dt.float32
    i16 = mybir.dt.int16

    nc = bacc.Bacc("TRN2", target_bir_lowering=False, debug=False,
                   num_swdge_queues=NQ, dynamic_dma_scratch_size=16384)

    xt = nc.declare_dram_parameter("xt", [P, NXP], f16, isOutput=False)
    wt = nc.declare_dram_parameter("wt", [D_IN, D_OUT], f16, isOutput=False)
    idx = nc.declare_dram_parameter("idx", [P, Ctot * 8], i16, isOutput=False)
    dloc = nc.declare_dram_parameter("dloc", [P, Ctot], f16, isOutput=False)
    iota = nc.declare_dram_parameter("iota", [P, P], f16, isOutput=False)
    recip = nc.declare_dram_parameter("recip", [P, NBLK], f32, isOutput=False)
    outF = nc.declare_dram_parameter("outF", [P, NBLK * D_OUT], f32, isOutput=True)
    if has_bias:
        brow = nc.declare_dram_parameter("brow", [1, D_OUT], f16, isOutput=False)
        ones = nc.declare_dram_parameter("ones", [1, P], f16, isOutput=False)

    def bcast3(ap, reps):
        # [P, mid] -> [P, mid, reps] via zero-stride inner dim
        return bass.AP(tensor=ap.tensor, offset=ap.offset,
                       ap=[ap.ap[0], ap.ap[1], [0, reps]])

    def rep_mid(ap, reps):
        # [P, n] -> [P, reps, n] via zero-stride middle dim
        return bass.AP(tensor=ap.tensor, offset=ap.offset,
                       ap=[ap.ap[0], [0, reps], ap.ap[1]])

    with tile.TileContext(nc) as tc, ExitStack() as ctx:
        consts = ctx.enter_context(tc.tile_pool(name="consts", bufs=1))
        dram = ctx.enter_context(tc.tile_pool(name="dram", bufs=1, space="DRAM"))
        xslabs = ctx.enter_context(tc.tile_pool(name="xslabs", bufs=2))
        ph1ps = ctx.enter_context(tc.tile_pool(name="ph1ps", bufs=2, space="PSUM"))
        hstage = ctx.enter_context(tc.tile_pool(name="hstage", bufs=3))
        ghp = ctx.enter_context(tc.tile_pool(name="ghp", bufs=2))
        ohp = ctx.enter_context(tc.tile_pool(name="ohp", bufs=2))
        aggps = ctx.enter_context(tc.tile_pool(name="aggps", bufs=4, space="PSUM"))
        outsp = ctx.enter_context(tc.tile_pool(name="outsp", bufs=3))

        s_wt = consts.tile([D_IN, D_OUT], f16)
        s_iota = consts.tile([P, P], f16)
        s_idx = consts.tile([P, Ctot * 8], i16)
        s_dloc = consts.tile([P, Ctot], f16)
        s_recip = consts.tile([P, NBLK], f32)
        nc.sync.dma_start(out=s_wt[:], in_=wt[:])
        nc.sync.dma_start(out=s_iota[:], in_=iota[:])
        nc.sync.dma_start(out=s_idx[:], in_=idx[:])
        nc.sync.dma_start(out=s_dloc[:], in_=dloc[:])
        nc.sync.dma_start(out=s_recip[:], in_=recip[:])
        if has_bias:
            s_brow = consts.tile([1, D_OUT], f16)
            s_ones = consts.tile([1, P], f16)
            nc.sync.dma_start(out=s_brow[:], in_=brow[:])
            nc.sync.dma_start(out=s_ones[:], in_=ones[:])

        # h scratch in DRAM, partition-major: node n -> flat 128B row
        # (n%128)*NCH + n//128
        h_l = dram.tile([P, NCH * D_OUT], f16)
        h_even = bass.AP(tensor=h_l[:].tensor, offset=0,
                         ap=[[2 * D_OUT, NPAIR], [1, D_OUT]])
        h_odd = bass.AP(tensor=h_l[:].tensor, offset=D_OUT,
                        ap=[[2 * D_OUT, NPAIR], [1, D_OUT]])

        # ---- Phase 1: h = x W^T (+ b) ----
        for s0 in range(0, NCH, G_SLAB):
            g = min(G_SLAB, NCH - s0)
            xs = xslabs.tile([P, G_SLAB * P], f16, tag="xs")
            nc.sync.dma_start(out=xs[:, : g * P], in_=xt[:, s0 * P : (s0 + g) * P])
            for h0 in range(0, g, G_H):
                gh_n = min(G_H, g - h0)
                ps = ph1ps.tile([P, G_H * D_OUT], f32, space="PSUM", tag="ph1")
                for c in range(gh_n):
                    nc.tensor.matmul(
                        ps[:, c * D_OUT : (c + 1) * D_OUT],
                        lhsT=xs[:, (h0 + c) * P : (h0 + c + 1) * P],
                        rhs=s_wt[:],
                        start=True,
                        stop=not has_bias,
                    )
                    if has_bias:
                        nc.tensor.matmul(
                            ps[:, c * D_OUT : (c + 1) * D_OUT],
                            lhsT=s_ones[:],
                            rhs=s_brow[:],
                            start=False,
                            stop=True,
                        )
                hs = hstage.tile([P, G_H * D_OUT], f16, tag="hs")
                nc.scalar.copy(out=hs[:, : gh_n * D_OUT], in_=ps[:, : gh_n * D_OUT])
                nc.sync.dma_start(
                    out=h_l[:, (s0 + h0) * D_OUT : (s0 + h0 + gh_n) * D_OUT],
                    in_=hs[:, : gh_n * D_OUT],
                )

        # ---- Phase 2: gather h rows per edge, aggregate per dest block ----
        qctr = [0]
        off = 0   # chunk offset into idx/dloc streams
        for sb0 in range(0, NBLK, SBN):
            sbb = list(range(sb0, min(sb0 + SBN, NBLK)))
            nb = len(sbb)
            ca = [CA[b] for b in sbb]
            cb = [CB[b] for b in sbb]
            csa = sum(ca)
            csb_o = sum(cb)
            csb = csa + csb_o

            gh = ghp.tile([P, csb, D_OUT], f16, tag="gh")
            for (c0, cc, src) in ((0, csa, h_even), (csa, csb_o, h_odd)):
                while cc > 0:
                    seg = min(cc, 48)
                    _gather128(
                        nc,
                        gh[:, c0 : c0 + seg, :],
                        src,
                        s_idx[:, (off + c0) * 8 : (off + c0 + seg) * 8],
                        seg * P,
                        qctr[0] % NQ,
                    )
                    qctr[0] += 1
                    c0 += seg
                    cc -= seg

            oh = ohp.tile([P, csb, P], f16, tag="oh")
            nc.vector.tensor_tensor(
                out=oh[:],
                in0=bcast3(s_dloc[:, off : off + csb], P),
                in1=rep_mid(s_iota[:], csb),
                op=mybir.AluOpType.is_equal,
            )

            agg = aggps.tile([P, nb * D_OUT], f32, space="PSUM", tag="agg")
            aoff = 0
            boff = csa
            for bi, b in enumerate(sbb):
                nchunks = ca[bi] + cb[bi]
                j = 0
                for c in range(ca[bi]):
                    nc.tensor.matmul(
                        agg[:, bi * D_OUT : (bi + 1) * D_OUT],
                        lhsT=oh[:, aoff + c, :],
                        rhs=gh[:, aoff + c, :],
                        start=(j == 0),
                        stop=(j == nchunks - 1),
                    )
                    j += 1
                for c in range(cb[bi]):
                    nc.tensor.matmul(
                        agg[:, bi * D_OUT : (bi + 1) * D_OUT],
                        lhsT=oh[:, boff + c, :],
                        rhs=gh[:, boff + c, :],
                        start=(j == 0),
                        stop=(j == nchunks - 1),
                    )
                    j += 1
                aoff += ca[bi]
                boff += cb[bi]

            outs = outsp.tile([P, nb * D_OUT], f32, tag="outs")
            for bi in range(nb):
                rb = s_recip[:, sb0 + bi : sb0 + bi + 1]
                nc.vector.tensor_tensor(
                    out=outs[:, bi * D_OUT : (bi + 1) * D_OUT],
                    in0=agg[:, bi * D_OUT : (bi + 1) * D_OUT],
                    in1=bass.AP(tensor=rb.tensor, offset=rb.offset,
                                ap=[rb.ap[0], [0, D_OUT]]),
                    op=mybir.AluOpType.mult,
                )
            nc.sync.dma_start(
                out=outF[:, sb0 * D_OUT : (sb0 + nb) * D_OUT], in_=outs[:]
            )
            off += csb

    nc.compile()
    return nc


def _wrap_idx(idx_list):
    """[n] int16 -> [128, n//16] wrapped + replicated layout."""
    n = idx_list.shape[0]
    w16 = idx_list.reshape(n // 16, 16).T  # [16, n/16]
    return np.tile(w16, (8, 1)).astype(np.int16)


def _host_prep(x, W, b, row, col):
    deg = np.bincount(row, minlength=N_NODES)
    recip = (1.0 / np.maximum(deg, 1)).astype(np.float32)

    # sort edges by (core, block, source-chunk parity)
    core = row // NPC
    local = row - core * NPC
    blk = local // P
    dl_all = (local - blk * P).astype(np.float16)
    par = (col // P) % 2
    key = (core * NBLK + blk) * 2 + par
    order = np.argsort(key, kind="stable")
    cs = col[order]
    dl = dl_all[order]

    counts = np.bincount(key, minlength=N_CORES * NBLK * 2).reshape(N_CORES, NBLK, 2)
    chunks = -(-counts // P)  # ceil
    CA = chunks[:, :, 0].max(axis=0)
    CB = chunks[:, :, 1].max(axis=0)
    both_zero = (CA + CB) == 0
    CA = np.where(both_zero, 1, CA)
    Ctot = int(CA.sum() + CB.sum())

    starts = np.zeros(N_CORES * NBLK * 2 + 1, np.int64)
    np.cumsum(counts.reshape(-1), out=starts[1:])

    # gather pair-index: node n -> (n%128)*(NCH//2) + (n//128)//2, int16-safe
    hidx = ((cs % P) * (NCH // 2) + (cs // P) // 2).astype(np.int16)

    idx_dev = np.zeros((N_CORES, P, Ctot * 8), np.int16)
    dloc_dev = np.zeros((N_CORES, P, Ctot), np.float16)
    recip_dev = np.zeros((N_CORES, P, NBLK), np.float32)

    # per-superblock chunk layout: [A-chunks of blocks | B-chunks of blocks]
    sb_ranges = []  # (blocks, [(par, b, chunk_start_in_stream)...])
    for k in range(N_CORES):
        idx_stream = np.zeros(Ctot * P, np.int16)
        dl_stream = np.full(Ctot * P, -1.0, np.float16)
        off = 0  # chunks
        for sb0 in range(0, NBLK, SBN):
            sbb = range(sb0, min(sb0 + SBN, NBLK))
            for par_i, CC in ((0, CA), (1, CB)):
                for bb in sbb:
                    gidx = (k * NBLK + bb) * 2 + par_i
                    s, e = starts[gidx], starts[gidx + 1]
                    o = off * P
                    idx_stream[o : o + (e - s)] = hidx[s:e]
                    dl_stream[o : o + (e - s)] = dl[s:e]
                    off += int(CC[bb])
        assert off == Ctot
        idx_dev[k] = _wrap_idx(idx_stream)
        dloc_dev[k] = dl_stream.reshape(Ctot, P).T
        base = k * NPC
        rr = np.zeros(NPAD, np.float32)
        rr[:NPC] = recip[base : base + NPC]
        recip_dev[k] = rr.reshape(NBLK, P).T

    xt = np.zeros((P, NXP), np.float16)
    xt[:, :N_NODES] = x.T.astype(np.float16)
    wt = np.ascontiguousarray(W.T).astype(np.float16)
    iota_t = np.tile(np.arange(P, dtype=np.float16), (P, 1))
    has_bias = bool(np.any(b))

    in_maps = []
    for k in range(N_CORES):
        m = dict(xt=xt, wt=wt, idx=idx_dev[k], dloc=dloc_dev[k],
                 iota=iota_t, recip=recip_dev[k])
        if has_bias:
            m["brow"] = b[None, :].astype(np.float16)
            m["ones"] = np.ones((1, P), np.float16)
        in_maps.append(m)
    return CA, CB, has_bias, in_maps


def kernel(x, W, b, row, col):
    global last_results
    x = np.asarray(x, dtype=np.float32)
    W = np.asarray(W, dtype=np.float32)
    b = np.asarray(b, dtype=np.float32)
    row = np.asarray(row).astype(np.int64)
    col = np.asarray(col).astype(np.int64)

    CA, CB, has_bias, in_maps = _host_prep(x, W, b, row, col)

    cache_key = (tuple(CA.tolist()), tuple(CB.tolist()), has_bias)
    if cache_key not in _prog_cache:
        _prog_cache[cache_key] = _build_program(CA, CB, has_bias)
    nc = _prog_cache[cache_key]

    res = run_bass_kernel_spmd(nc, in_maps, core_ids=list(range(N_CORES)))
    last_results = res

    out = np.empty((N_NODES, D_OUT), np.float32)
    for k in range(N_CORES):
        of = res.results[k]["outF"].reshape(P, NBLK, D_OUT)
        out[k * NPC : (k + 1) * NPC] = of.transpose(1, 0, 2).reshape(NPAD, D_OUT)[:NPC]
    return out


# revision 13
# speedup vs baseline: 1.0421x; 1.0421x over previous
"""GNN mean-aggregator (h = xW^T + b; out[i] = mean_{(i,j) in E} h[j]) on 8 trn2 cores.

Strategy (graph/data parallel over destination nodes):
  - Each core owns a contiguous range of 6250 destination nodes.
  - Host sorts edges by destination block, splits by source-node half
    (int16 gather index limit), pads each (block, half) group to whole
    128-edge chunks uniformly across cores (SPMD: one program).
  - Device: dma_gather fetches fp16 x rows per edge in a few large calls
    (descriptor generation on the GpSimd/Q7 engine is the bottleneck at
    ~2ns/edge; large calls amortize the ~1us/call fixed cost), the
    host-precomputed fp8 one-hot [edge, dest-local] matrix streams from
    HBM (no DVE is_equal on the critical path), and TensorE matmuls
    accumulate sum_{e} x[col_e] per destination block in PSUM
    (feature-major).  A second small matmul applies W^T, then the result
    is scaled by 1/deg and written out.
"""
import sys

sys.path.insert(0, "/opt/trn_rl_repo")

from contextlib import ExitStack

import ml_dtypes
import numpy as np

from concourse import bass, bacc, mybir, tile
from concourse.bass_utils import run_bass_kernel_spmd

N_NODES = 50000
N_EDGES = 800000
D_IN = 128
D_OUT = 64
N_CORES = 8
NPC = N_NODES // N_CORES      # 6250 destination nodes per core
P = 128
NBLK = (NPC + P - 1) // P     # 49 blocks of 128 destinations
NPAD = NBLK * P               # 6272 padded destinations
HALF = 32768                  # int16 gather-index boundary
SBN = 8                       # blocks per superblock

_prog_cache = {}
last_results = None  # test harness introspection


def _build_program(CA, CB, has_bias):
    """CA/CB: per-block chunk counts for lo/hi source half (uniform across
    cores)."""
    CA = [int(v) for v in CA]
    CB = [int(v) for v in CB]
    CAtot = sum(CA)
    CBtot = sum(CB)
    Ctot = CAtot + CBtot

    nc = bacc.Bacc("TRN2", target_bir_lowering=False, debug=False,
                   num_swdge_queues=4, dynamic_dma_scratch_size=16384)
    f16 = mybir.dt.float16
    f32 = mybir.dt.float32
    f8 = mybir.dt.float8e4
    i16 = mybir.dt.int16

    xlo = nc.declare_dram_parameter("xlo", [HALF, D_IN], f16, isOutput=False)
    xhi = nc.declare_dram_parameter("xhi", [N_NODES - HALF, D_IN], f16, isOutput=False)
    idx = nc.declare_dram_parameter("idx", [P, Ctot * 8], i16, isOutput=False)
    ohm = nc.declare_dram_parameter("ohm", [P, Ctot * P], f8, isOutput=False)
    wt = nc.declare_dram_parameter("wt", [D_IN, D_OUT], f16, isOutput=False)
    scale = nc.declare_dram_parameter("scale", [D_OUT, NPAD], f16, isOutput=False)
    outT = nc.declare_dram_parameter("outT", [D_OUT, NPAD], f32, isOutput=True)
    if has_bias:
        biasr = nc.declare_dram_parameter("biasr", [D_OUT, NPAD], f32, isOutput=False)

    with tile.TileContext(nc) as tc, ExitStack() as ctx:
        consts = ctx.enter_context(tc.tile_pool(name="consts", bufs=1))
        gxp = ctx.enter_context(tc.tile_pool(name="gxp", bufs=3))
        ohp = ctx.enter_context(tc.tile_pool(name="ohp", bufs=3))
        aggsb = ctx.enter_context(tc.tile_pool(name="aggsb", bufs=3))
        outsb = ctx.enter_context(tc.tile_pool(name="outsb", bufs=3))
        aggps = ctx.enter_context(tc.tile_pool(name="aggps", bufs=2, space="PSUM"))
        projps = ctx.enter_context(tc.tile_pool(name="projps", bufs=2, space="PSUM"))

        s_wt = consts.tile([D_IN, D_OUT], f16)
        s_idx = consts.tile([P, Ctot * 8], i16)
        s_scale = consts.tile([D_OUT, NPAD], f16)
        nc.sync.dma_start(out=s_wt[:], in_=wt[:])
        nc.sync.dma_start(out=s_idx[:], in_=idx[:])
        nc.sync.dma_start(out=s_scale[:], in_=scale[:])
        if has_bias:
            s_bias = consts.tile([D_OUT, NPAD], f32)
            nc.sync.dma_start(out=s_bias[:], in_=biasr[:])

        qctr = [0]
        offA = 0               # chunk offset within the A (lo) idx stream
        offB = 0
        for sb0 in range(0, NBLK, SBN):
            sbb = list(range(sb0, min(sb0 + SBN, NBLK)))
            nb = len(sbb)
            ca = [CA[b] for b in sbb]
            cb = [CB[b] for b in sbb]
            casb = sum(ca)
            cbsb = sum(cb)
            csb = casb + cbsb

            # one large gather call per (superblock, half)
            gx = gxp.tile([P, csb, D_IN], f16, tag="gx")
            for (c0, cc, src, woff) in ((0, casb, xlo, offA),
                                        (casb, cbsb, xhi, CAtot + offB)):
                if cc == 0:
                    continue
                nc.gpsimd.dma_gather(
                    gx[:, c0 : c0 + cc, :], src[:],
                    s_idx[:, woff * 8 : (woff + cc) * 8],
                    cc * P, cc * P, D_IN, single_packet=False,
                    queue_num=qctr[0] % 4,
                )
                qctr[0] += 1

            # host-precomputed one-hot slab (fp8): [P, csb, P]
            # host layout: [all A-stream chunks | all B-stream chunks]
            oh = ohp.tile([P, csb, P], f8, tag="oh")
            oh2d = bass.AP(tensor=oh[:].tensor, offset=oh[:].offset,
                           ap=[oh[:].ap[0], [1, csb * P]])
            if casb > 0:
                nc.sync.dma_start(
                    out=bass.AP(tensor=oh2d.tensor, offset=oh2d.offset,
                                ap=[oh2d.ap[0], [1, casb * P]]),
                    in_=ohm[:, offA * P : (offA + casb) * P],
                )
            if cbsb > 0:
                nc.sync.dma_start(
                    out=bass.AP(tensor=oh2d.tensor, offset=oh2d.offset + casb * P,
                                ap=[oh2d.ap[0], [1, cbsb * P]]),
                    in_=ohm[:, (CAtot + offB) * P : (CAtot + offB + cbsb) * P],
                )

            agg_ps = aggps.tile([P, nb * P], f32, space="PSUM", tag="aggps")
            a0 = 0
            b0 = casb
            for bi in range(nb):
                nchunks = ca[bi] + cb[bi]
                j = 0
                for c in range(ca[bi]):
                    nc.tensor.matmul(
                        agg_ps[:, bi * P : (bi + 1) * P],
                        lhsT=gx[:, a0 + c, :],
                        rhs=oh[:, a0 + c, :],
                        start=(j == 0),
                        stop=(j == nchunks - 1),
                    )
                    j += 1
                for c in range(cb[bi]):
                    nc.tensor.matmul(
                        agg_ps[:, bi * P : (bi + 1) * P],
                        lhsT=gx[:, b0 + c, :],
                        rhs=oh[:, b0 + c, :],
                        start=(j == 0),
                        stop=(j == nchunks - 1),
                    )
                    j += 1
                a0 += ca[bi]
                b0 += cb[bi]

            agg_s = aggsb.tile([P, nb * P], f16, tag="aggsb")
            nc.scalar.copy(out=agg_s[:], in_=agg_ps[:])

            proj_ps = projps.tile([D_OUT, nb * P], f32, space="PSUM", tag="projps")
            # one matmul per PSUM bank (512 fp32 cols) to avoid bank crossing
            for p0 in range(0, nb * P, 512):
                pw = min(512, nb * P - p0)
                nc.tensor.matmul(proj_ps[:, p0 : p0 + pw],
                                 lhsT=s_wt[:], rhs=agg_s[:, p0 : p0 + pw],
                                 start=True, stop=True)

            out_s = outsb.tile([D_OUT, nb * P], f32, tag="outsb")
            colsl = slice(sb0 * P, sb0 * P + nb * P)
            nc.vector.tensor_tensor(out=out_s[:], in0=proj_ps[:],
                                    in1=s_scale[:, colsl], op=mybir.AluOpType.mult)
            if has_bias:
                nc.vector.tensor_tensor(out=out_s[:], in0=out_s[:],
                                        in1=s_bias[:, colsl], op=mybir.AluOpType.add)
            nc.sync.dma_start(out=outT[:, colsl], in_=out_s[:])

            offA += casb
            offB += cbsb

    nc.compile()
    return nc


def _wrap_idx(idx_list):
    """[n] int16 -> [128, n//16] wrapped + replicated layout."""
    n = idx_list.shape[0]
    w16 = idx_list.reshape(n // 16, 16).T  # [16, n/16]
    return np.tile(w16, (8, 1)).astype(np.int16)


def _host_prep(x, W, b, row, col):
    deg = np.bincount(row, minlength=N_NODES)
    recip = (1.0 / np.maximum(deg, 1)).astype(np.float32)
    mask = (deg > 0).astype(np.float32)

    # sort edges by (core, block, half)
    core = row // NPC
    local = row - core * NPC
    blk = local // P
    dloc = (local - blk * P).astype(np.int16)
    half = (col >= HALF).astype(np.int64)
    key = (core * NBLK + blk) * 2 + half
    order = np.argsort(key, kind="stable")
    cs = col[order]
    dl = dloc[order]

    counts = np.bincount(key, minlength=N_CORES * NBLK * 2).reshape(N_CORES, NBLK, 2)
    chunks = -(-counts // P)  # ceil
    CA = np.maximum(chunks[:, :, 0].max(axis=0), 1)  # [NBLK]
    CB = chunks[:, :, 1].max(axis=0)                 # [NBLK]
    CAtot = int(CA.sum())
    CBtot = int(CB.sum())
    Ctot = CAtot + CBtot

    starts = np.zeros(N_CORES * NBLK * 2 + 1, np.int64)
    np.cumsum(counts.reshape(-1), out=starts[1:])

    idx_dev = np.zeros((N_CORES, P, Ctot * 8), np.int16)
    oh_dev = np.zeros((N_CORES, P, Ctot * P), ml_dtypes.float8_e4m3fn)
    scale_dev = np.zeros((N_CORES, D_OUT, NPAD), np.float16)
    bias_dev = None
    has_bias = bool(np.any(b))
    if has_bias:
        bias_dev = np.zeros((N_CORES, D_OUT, NPAD), np.float32)

    jj = np.arange(P, dtype=np.int16)
    for k in range(N_CORES):
        # A-stream chunks first (all blocks), then B-stream
        idx_streamA = np.zeros(CAtot * P, np.int16)
        dl_streamA = np.full(CAtot * P, -1, np.int16)
        idx_streamB = np.zeros(max(CBtot, 1) * P, np.int16)
        dl_streamB = np.full(max(CBtot, 1) * P, -1, np.int16)
        offA = 0
        offB = 0
        for bidx in range(NBLK):
            for h, (CC, istream, dstream, base_sub, off) in enumerate((
                (CA, idx_streamA, dl_streamA, 0, offA),
                (CB, idx_streamB, dl_streamB, HALF, offB),
            )):
                gidx = (k * NBLK + bidx) * 2 + h
                s, e = starts[gidx], starts[gidx + 1]
                o = off * P
                istream[o : o + (e - s)] = (cs[s:e] - base_sub).astype(np.int16)
                dstream[o : o + (e - s)] = dl[s:e]
            offA += int(CA[bidx])
            offB += int(CB[bidx])
        idx_dev[k][:, : CAtot * 8] = _wrap_idx(idx_streamA)
        if CBtot > 0:
            idx_dev[k][:, CAtot * 8 :] = _wrap_idx(idx_streamB)
        # one-hot [P, Ctot, P]: oh[p, c, j] = (dl_stream[c*128+p] == j)
        dl_all = np.concatenate([dl_streamA, dl_streamB[: CBtot * P]])
        ohk = (dl_all[:, None] == jj[None, :]).astype(ml_dtypes.float8_e4m3fn)
        oh_dev[k] = ohk.reshape(Ctot, P, P).transpose(1, 0, 2).reshape(P, Ctot * P)
        base = k * NPC
        sc = np.zeros(NPAD, np.float32)
        sc[:NPC] = recip[base : base + NPC]
        scale_dev[k][:, :] = sc[None, :].astype(np.float16)
        if has_bias:
            bm = np.zeros(NPAD, np.float32)
            bm[:NPC] = mask[base : base + NPC]
            bias_dev[k][:, :] = b[:, None] * bm[None, :]

    xlo = np.ascontiguousarray(x[:HALF]).astype(np.float16)
    xhi = np.ascontiguousarray(x[HALF:]).astype(np.float16)
    wt = np.ascontiguousarray(W.T).astype(np.float16)

    in_maps = []
    for k in range(N_CORES):
        m = dict(xlo=xlo, xhi=xhi, idx=idx_dev[k], ohm=oh_dev[k],
                 wt=wt, scale=scale_dev[k])
        if has_bias:
            m["biasr"] = bias_dev[k]
        in_maps.append(m)
    return CA, CB, has_bias, in_maps


def kernel(x, W, b, row, col):
    global last_results
    x = np.asarray(x, dtype=np.float32)
    W = np.asarray(W, dtype=np.float32)
    b = np.asarray(b, dtype=np.float32)
    row = np.asarray(row).astype(np.int64)
    col = np.asarray(col).astype(np.int64)

    CA, CB, has_bias, in_maps = _host_prep(x, W, b, row, col)

    cache_key = (tuple(CA.tolist()), tuple(CB.tolist()), has_bias)
    if cache_key not in _prog_cache:
        _prog_cache[cache_key] = _build_program(CA, CB, has_bias)
    nc = _prog_cache[cache_key]

    res = run_bass_kernel_spmd(nc, in_maps, core_ids=list(range(N_CORES)))
    last_results = res

    out = np.empty((N_NODES, D_OUT), np.float32)
    for k in range(N_CORES):
        out[k * NPC : (k + 1) * NPC] = res.results[k]["outT"][:, :NPC].T
    return out


# revision 16
# speedup vs baseline: 1.1362x; 1.0903x over previous
"""GNN mean-aggregator (h = xW^T + b; out[i] = mean_{(i,j) in E} h[j]) on 8 trn2 cores.

Strategy (graph/data parallel over destination nodes):
  - Each core owns a contiguous range of 6250 destination nodes.
  - Host sorts edges by destination block, splits by source-node half
    (int16 gather index limit), pads each (block, half) group to whole
    128-edge chunks uniformly across cores (SPMD: one program).
  - Device: dma_gather fetches fp16 x rows per edge in a few large calls
    (descriptor generation on the GpSimd/Q7 engine is the bottleneck at
    ~2ns/edge; large calls amortize the ~1us/call fixed cost), the
    host-precomputed fp8 one-hot [edge, dest-local] matrix streams from
    HBM (no DVE is_equal on the critical path), and TensorE matmuls
    accumulate sum_{e} x[col_e] per destination block in PSUM
    (feature-major).  A second small matmul applies W^T, then the result
    is scaled by 1/deg and written out.
"""
import sys

sys.path.insert(0, "/opt/trn_rl_repo")

from contextlib import ExitStack

import ml_dtypes
import numpy as np

from concourse import bass, bacc, mybir, tile
from concourse.bass_utils import run_bass_kernel_spmd

N_NODES = 50000
N_EDGES = 800000
D_IN = 128
D_OUT = 64
N_CORES = 8
NPC = N_NODES // N_CORES      # 6250 destination nodes per core
P = 128
NBLK = (NPC + P - 1) // P     # 49 blocks of 128 destinations
NPAD = NBLK * P               # 6272 padded destinations
HALF = 32768                  # int16 gather-index boundary
SBN = 8                       # blocks per superblock

_prog_cache = {}
last_results = None  # test harness introspection


def _build_program(CA, CB, has_bias):
    """CA/CB: per-block chunk counts for lo/hi source half (uniform across
    cores)."""
    CA = [int(v) for v in CA]
    CB = [int(v) for v in CB]
    CAtot = sum(CA)
    CBtot = sum(CB)
    Ctot = CAtot + CBtot

    nc = bacc.Bacc("TRN2", target_bir_lowering=False, debug=False,
                   num_swdge_queues=4, dynamic_dma_scratch_size=65536)
    f16 = mybir.dt.float16
    f32 = mybir.dt.float32
    f8 = mybir.dt.float8e4
    i16 = mybir.dt.int16

    xlo = nc.declare_dram_parameter("xlo", [HALF, D_IN], f16, isOutput=False)
    xhi = nc.declare_dram_parameter("xhi", [N_NODES - HALF, D_IN], f16, isOutput=False)
    idx = nc.declare_dram_parameter("idx", [P, Ctot * 8], i16, isOutput=False)
    ohm = nc.declare_dram_parameter("ohm", [P, Ctot * P], f8, isOutput=False)
    wt = nc.declare_dram_parameter("wt", [D_IN, D_OUT], f16, isOutput=False)
    scale = nc.declare_dram_parameter("scale", [D_OUT, NPAD], f16, isOutput=False)
    outT = nc.declare_dram_parameter("outT", [D_OUT, NPAD], f32, isOutput=True)
    if has_bias:
        biasr = nc.declare_dram_parameter("biasr", [D_OUT, NPAD], f32, isOutput=False)

    with tile.TileContext(nc) as tc, ExitStack() as ctx:
        consts = ctx.enter_context(tc.tile_pool(name="consts", bufs=1))
        gxp = ctx.enter_context(tc.tile_pool(name="gxp", bufs=2))
        ohp = ctx.enter_context(tc.tile_pool(name="ohp", bufs=2))
        aggsb = ctx.enter_context(tc.tile_pool(name="aggsb", bufs=3))
        outsb = ctx.enter_context(tc.tile_pool(name="outsb", bufs=3))
        aggps = ctx.enter_context(tc.tile_pool(name="aggps", bufs=2, space="PSUM"))
        projps = ctx.enter_context(tc.tile_pool(name="projps", bufs=2, space="PSUM"))

        s_wt = consts.tile([D_IN, D_OUT], f16)
        s_idx = consts.tile([P, Ctot * 8], i16)
        s_scale = consts.tile([D_OUT, NPAD], f16)
        nc.sync.dma_start(out=s_wt[:], in_=wt[:])
        nc.sync.dma_start(out=s_idx[:], in_=idx[:])
        nc.sync.dma_start(out=s_scale[:], in_=scale[:])
        if has_bias:
            s_bias = consts.tile([D_OUT, NPAD], f32)
            nc.sync.dma_start(out=s_bias[:], in_=biasr[:])

        qctr = [0]
        offA = 0               # chunk offset within the A (lo) idx stream
        offB = 0
        for sb0 in range(0, NBLK, SBN):
            sbb = list(range(sb0, min(sb0 + SBN, NBLK)))
            nb = len(sbb)
            ca = [CA[b] for b in sbb]
            cb = [CB[b] for b in sbb]
            casb = sum(ca)
            cbsb = sum(cb)
            csb = casb + cbsb

            # gather calls capped at 32 chunks to stay well under ring capacity
            gx = gxp.tile([P, csb, D_IN], f16, tag="gx")
            for (c0, cc, src, woff) in ((0, casb, xlo, offA),
                                        (casb, cbsb, xhi, CAtot + offB)):
                nseg = -(-cc // 32) if cc else 0
                s0 = 0
                for g in range(nseg):
                    seg = (cc - s0) // (nseg - g)
                    nc.gpsimd.dma_gather(
                        gx[:, c0 + s0 : c0 + s0 + seg, :], src[:],
                        s_idx[:, (woff + s0) * 8 : (woff + s0 + seg) * 8],
                        seg * P, seg * P, D_IN, single_packet=False,
                        queue_num=qctr[0] % 4,
                    )
                    qctr[0] += 1
                    s0 += seg

            # host-precomputed one-hot slab (fp8): [P, csb, P]
            # host layout: [all A-stream chunks | all B-stream chunks]
            oh = ohp.tile([P, csb, P], f8, tag="oh")
            oh2d = bass.AP(tensor=oh[:].tensor, offset=oh[:].offset,
                           ap=[oh[:].ap[0], [1, csb * P]])
            if casb > 0:
                nc.sync.dma_start(
                    out=bass.AP(tensor=oh2d.tensor, offset=oh2d.offset,
                                ap=[oh2d.ap[0], [1, casb * P]]),
                    in_=ohm[:, offA * P : (offA + casb) * P],
                )
            if cbsb > 0:
                nc.sync.dma_start(
                    out=bass.AP(tensor=oh2d.tensor, offset=oh2d.offset + casb * P,
                                ap=[oh2d.ap[0], [1, cbsb * P]]),
                    in_=ohm[:, (CAtot + offB) * P : (CAtot + offB + cbsb) * P],
                )

            agg_ps = aggps.tile([P, nb * P], f32, space="PSUM", tag="aggps")
            a0 = 0
            b0 = casb
            for bi in range(nb):
                nchunks = ca[bi] + cb[bi]
                j = 0
                for c in range(ca[bi]):
                    nc.tensor.matmul(
                        agg_ps[:, bi * P : (bi + 1) * P],
                        lhsT=gx[:, a0 + c, :],
                        rhs=oh[:, a0 + c, :],
                        start=(j == 0),
                        stop=(j == nchunks - 1),
                    )
                    j += 1
                for c in range(cb[bi]):
                    nc.tensor.matmul(
                        agg_ps[:, bi * P : (bi + 1) * P],
                        lhsT=gx[:, b0 + c, :],
                        rhs=oh[:, b0 + c, :],
                        start=(j == 0),
                        stop=(j == nchunks - 1),
                    )
                    j += 1
                a0 += ca[bi]
                b0 += cb[bi]

            agg_s = aggsb.tile([P, nb * P], f16, tag="aggsb")
            nc.scalar.copy(out=agg_s[:], in_=agg_ps[:])

            proj_ps = projps.tile([D_OUT, nb * P], f32, space="PSUM", tag="projps")
            # one matmul per PSUM bank (512 fp32 cols) to avoid bank crossing
            for p0 in range(0, nb * P, 512):
                pw = min(512, nb * P - p0)
                nc.tensor.matmul(proj_ps[:, p0 : p0 + pw],
                                 lhsT=s_wt[:], rhs=agg_s[:, p0 : p0 + pw],
                                 start=True, stop=True)

            out_s = outsb.tile([D_OUT, nb * P], f32, tag="outsb")
            colsl = slice(sb0 * P, sb0 * P + nb * P)
            nc.vector.tensor_tensor(out=out_s[:], in0=proj_ps[:],
                                    in1=s_scale[:, colsl], op=mybir.AluOpType.mult)
            if has_bias:
                nc.vector.tensor_tensor(out=out_s[:], in0=out_s[:],
                                        in1=s_bias[:, colsl], op=mybir.AluOpType.add)
            nc.sync.dma_start(out=outT[:, colsl], in_=out_s[:])

            offA += casb
            offB += cbsb

    nc.compile()
    return nc


def _wrap_idx(idx_list):
    """[n] int16 -> [128, n//16] wrapped + replicated layout."""
    n = idx_list.shape[0]
    w16 = idx_list.reshape(n // 16, 16).T  # [16, n/16]
    return np.tile(w16, (8, 1)).astype(np.int16)


def _host_prep(x, W, b, row, col):
    deg = np.bincount(row, minlength=N_NODES)
    recip = (1.0 / np.maximum(deg, 1)).astype(np.float32)
    mask = (deg > 0).astype(np.float32)

    # sort edges by (core, block, half)
    core = row // NPC
    local = row - core * NPC
    blk = local // P
    dloc = (local - blk * P).astype(np.int16)
    half = (col >= HALF).astype(np.int64)
    key = (core * NBLK + blk) * 2 + half
    order = np.argsort(key, kind="stable")
    cs = col[order]
    dl = dloc[order]

    counts = np.bincount(key, minlength=N_CORES * NBLK * 2).reshape(N_CORES, NBLK, 2)
    chunks = -(-counts // P)  # ceil
    CA = np.maximum(chunks[:, :, 0].max(axis=0), 1)  # [NBLK]
    CB = chunks[:, :, 1].max(axis=0)                 # [NBLK]
    CAtot = int(CA.sum())
    CBtot = int(CB.sum())
    Ctot = CAtot + CBtot

    starts = np.zeros(N_CORES * NBLK * 2 + 1, np.int64)
    np.cumsum(counts.reshape(-1), out=starts[1:])

    idx_dev = np.zeros((N_CORES, P, Ctot * 8), np.int16)
    oh_dev = np.zeros((N_CORES, P, Ctot * P), ml_dtypes.float8_e4m3fn)
    scale_dev = np.zeros((N_CORES, D_OUT, NPAD), np.float16)
    bias_dev = None
    has_bias = bool(np.any(b))
    if has_bias:
        bias_dev = np.zeros((N_CORES, D_OUT, NPAD), np.float32)

    jj = np.arange(P, dtype=np.int16)
    for k in range(N_CORES):
        # A-stream chunks first (all blocks), then B-stream
        idx_streamA = np.zeros(CAtot * P, np.int16)
        dl_streamA = np.full(CAtot * P, -1, np.int16)
        idx_streamB = np.zeros(max(CBtot, 1) * P, np.int16)
        dl_streamB = np.full(max(CBtot, 1) * P, -1, np.int16)
        offA = 0
        offB = 0
        for bidx in range(NBLK):
            for h, (CC, istream, dstream, base_sub, off) in enumerate((
                (CA, idx_streamA, dl_streamA, 0, offA),
                (CB, idx_streamB, dl_streamB, HALF, offB),
            )):
                gidx = (k * NBLK + bidx) * 2 + h
                s, e = starts[gidx], starts[gidx + 1]
                o = off * P
                istream[o : o + (e - s)] = (cs[s:e] - base_sub).astype(np.int16)
                dstream[o : o + (e - s)] = dl[s:e]
            offA += int(CA[bidx])
            offB += int(CB[bidx])
        idx_dev[k][:, : CAtot * 8] = _wrap_idx(idx_streamA)
        if CBtot > 0:
            idx_dev[k][:, CAtot * 8 :] = _wrap_idx(idx_streamB)
        # one-hot [P, Ctot, P]: oh[p, c, j] = (dl_stream[c*128+p] == j)
        dl_all = np.concatenate([dl_streamA, dl_streamB[: CBtot * P]])
        ohk = (dl_all[:, None] == jj[None, :]).astype(ml_dtypes.float8_e4m3fn)
        oh_dev[k] = ohk.reshape(Ctot, P, P).transpose(1, 0, 2).reshape(P, Ctot * P)
        base = k * NPC
        sc = np.zeros(NPAD, np.float32)
        sc[:NPC] = recip[base : base + NPC]
        scale_dev[k][:, :] = sc[None, :].astype(np.float16)
        if has_bias:
            bm = np.zeros(NPAD, np.float32)
            bm[:NPC] = mask[base : base + NPC]
            bias_dev[k][:, :] = b[:, None] * bm[None, :]

    xlo = np.ascontiguousarray(x[:HALF]).astype(np.float16)
    xhi = np.ascontiguousarray(x[HALF:]).astype(np.float16)
    wt = np.ascontiguousarray(W.T).astype(np.float16)

    in_maps = []
    for k in range(N_CORES):
        m = dict(xlo=xlo, xhi=xhi, idx=idx_dev[k], ohm=oh_dev[k],
                 wt=wt, scale=scale_dev[k])
        if has_bias:
            m["biasr"] = bias_dev[k]
        in_maps.append(m)
    return CA, CB, has_bias, in_maps


def kernel(x, W, b, row, col):
    global last_results
    x = np.asarray(x, dtype=np.float32)
    W = np.asarray(W, dtype=np.float32)
    b = np.asarray(b, dtype=np.float32)
    row = np.asarray(row).astype(np.int64)
    col = np.asarray(col).astype(np.int64)

    CA, CB, has_bias, in_maps = _host_prep(x, W, b, row, col)

    cache_key = (tuple(CA.tolist()), tuple(CB.tolist()), has_bias)
    if cache_key not in _prog_cache:
        _prog_cache[cache_key] = _build_program(CA, CB, has_bias)
    nc = _prog_cache[cache_key]

    res = run_bass_kernel_spmd(nc, in_maps, core_ids=list(range(N_CORES)))
    last_results = res

    out = np.empty((N_NODES, D_OUT), np.float32)
    for k in range(N_CORES):
        out[k * NPC : (k + 1) * NPC] = res.results[k]["outT"][:, :NPC].T
    return out


# revision 18
# speedup vs baseline: 1.1978x; 1.0543x over previous
"""GNN mean-aggregator (h = xW^T + b; out[i] = mean_{(i,j) in E} h[j]) on 8 trn2 cores.

Strategy (graph/data parallel over destination nodes):
  - Each core owns a contiguous range of 6250 destination nodes.
  - Host sorts edges by destination, groups them into 128-destination blocks,
    splits each block's edges by source-node half (int16 gather index limit),
    and pads each (block, half) group to whole 128-edge chunks, uniformly
    across cores (SPMD: one program, per-core data).
  - Device: dma_gather fetches fp16 x rows per edge (edge-major chunks),
    a one-hot matrix built with a single broadcast is_equal per gather maps
    edges to their local destination, and TensorE matmuls accumulate
    sum_{e} x[col_e] per destination block in PSUM (feature-major).
    A second small matmul applies W^T, then the result is scaled by 1/deg
    (and bias, masked for deg=0) and written out.
"""
import sys

sys.path.insert(0, "/opt/trn_rl_repo")

from contextlib import ExitStack

import numpy as np

from concourse import bass, bacc, mybir, tile
from concourse.bass_utils import run_bass_kernel_spmd

N_NODES = 50000
N_EDGES = 800000
D_IN = 128
D_OUT = 64
N_CORES = 8
NPC = N_NODES // N_CORES      # 6250 destination nodes per core
P = 128
NBLK = (NPC + P - 1) // P     # 49 blocks of 128 destinations
NPAD = NBLK * P               # 6272 padded destinations
HALF = 32768                  # int16 gather-index boundary
SB = 4                        # blocks per superblock (gather granularity)
NSB = (NBLK + SB - 1) // SB   # 13 superblocks

_prog_cache = {}
last_results = None  # test harness introspection


def _build_program(CA, CB, has_bias):
    """CA/CB: per-block chunk counts (uniform across cores)."""
    CA = list(CA)
    CB = list(CB)
    CAtot = sum(CA)
    CBtot = sum(CB)

    nc = bacc.Bacc("TRN2", target_bir_lowering=False, debug=False,
                   num_swdge_queues=4, dynamic_dma_scratch_size=16384)
    f16 = mybir.dt.float16
    f32 = mybir.dt.float32
    i16 = mybir.dt.int16

    xlo = nc.declare_dram_parameter("xlo", [HALF, D_IN], f16, isOutput=False)
    xhi = nc.declare_dram_parameter("xhi", [N_NODES - HALF, D_IN], f16, isOutput=False)
    idxA = nc.declare_dram_parameter("idxA", [P, CAtot * 8], i16, isOutput=False)
    idxB = nc.declare_dram_parameter("idxB", [P, max(CBtot, 1) * 8], i16, isOutput=False)
    dlocA = nc.declare_dram_parameter("dlocA", [P, CAtot], f16, isOutput=False)
    dlocB = nc.declare_dram_parameter("dlocB", [P, max(CBtot, 1)], f16, isOutput=False)
    iota = nc.declare_dram_parameter("iota", [P, P], f16, isOutput=False)
    wt = nc.declare_dram_parameter("wt", [D_IN, D_OUT], f16, isOutput=False)
    scale = nc.declare_dram_parameter("scale", [D_OUT, NPAD], f16, isOutput=False)
    if has_bias:
        biasr = nc.declare_dram_parameter("biasr", [D_OUT, NPAD], f32, isOutput=False)
    outT = nc.declare_dram_parameter("outT", [D_OUT, NPAD], f32, isOutput=True)

    def bcast_mid(ap, reps):
        # [P, C] -> [P, C, reps] via zero-stride inner dim
        return bass.AP(tensor=ap.tensor, offset=ap.offset,
                       ap=[ap.ap[0], ap.ap[1], [0, reps]])

    def rep_mid(ap, reps):
        # [P, n] -> [P, reps, n] via zero-stride middle dim
        return bass.AP(tensor=ap.tensor, offset=ap.offset,
                       ap=[ap.ap[0], [0, reps], ap.ap[1]])

    with tile.TileContext(nc) as tc, ExitStack() as ctx:
        consts = ctx.enter_context(tc.tile_pool(name="consts", bufs=1))
        gxpA = ctx.enter_context(tc.tile_pool(name="gxA", bufs=4))
        gxpB = ctx.enter_context(tc.tile_pool(name="gxB", bufs=4))
        ohpA = ctx.enter_context(tc.tile_pool(name="ohA", bufs=4))
        ohpB = ctx.enter_context(tc.tile_pool(name="ohB", bufs=4))
        aggsb = ctx.enter_context(tc.tile_pool(name="aggsb", bufs=3))
        outsb = ctx.enter_context(tc.tile_pool(name="outsb", bufs=3))
        aggps = ctx.enter_context(tc.tile_pool(name="aggps", bufs=3, space="PSUM"))
        projps = ctx.enter_context(tc.tile_pool(name="projps", bufs=2, space="PSUM"))

        s_iota = consts.tile([P, P], f16)
        s_wt = consts.tile([D_IN, D_OUT], f16)
        s_idxA = consts.tile([P, CAtot * 8], i16)
        s_idxB = consts.tile([P, max(CBtot, 1) * 8], i16)
        s_dlocA = consts.tile([P, CAtot], f16)
        s_dlocB = consts.tile([P, max(CBtot, 1)], f16)
        s_scale = consts.tile([D_OUT, NPAD], f16)
        nc.sync.dma_start(out=s_iota[:], in_=iota[:])
        nc.sync.dma_start(out=s_wt[:], in_=wt[:])
        nc.sync.dma_start(out=s_idxA[:], in_=idxA[:])
        nc.sync.dma_start(out=s_idxB[:], in_=idxB[:])
        nc.sync.dma_start(out=s_dlocA[:], in_=dlocA[:])
        nc.sync.dma_start(out=s_dlocB[:], in_=dlocB[:])
        nc.sync.dma_start(out=s_scale[:], in_=scale[:])
        if has_bias:
            s_bias = consts.tile([D_OUT, NPAD], f32)
            nc.sync.dma_start(out=s_bias[:], in_=biasr[:])

        offA = 0
        offB = 0
        qctr = [0]
        for sb in range(NSB):
            blocks = list(range(sb * SB, min(sb * SB + SB, NBLK)))
            nb = len(blocks)
            ca = [CA[b] for b in blocks]
            cb = [CB[b] for b in blocks]
            casb = sum(ca)
            cbsb = sum(cb)

            gxA = gxpA.tile([P, casb, D_IN], f16, tag="gxA")
            nsegA = -(-casb // 16)
            s0 = 0
            for g in range(nsegA):
                seg = (casb - s0) // (nsegA - g)
                nc.gpsimd.dma_gather(
                    gxA[:, s0 : s0 + seg, :], xlo[:],
                    s_idxA[:, (offA + s0) * 8 : (offA + s0 + seg) * 8],
                    seg * P, seg * P, D_IN, single_packet=False,
                    queue_num=qctr[0] % 4,
                )
                qctr[0] += 1
                s0 += seg
            ohA = ohpA.tile([P, casb, P], f16, tag="ohA")
            nc.vector.tensor_tensor(
                out=ohA[:],
                in0=bcast_mid(s_dlocA[:, offA : offA + casb], P),
                in1=rep_mid(s_iota[:], casb),
                op=mybir.AluOpType.is_equal,
            )
            if cbsb > 0:
                gxB = gxpB.tile([P, cbsb, D_IN], f16, tag="gxB")
                nsegB = -(-cbsb // 16)
                s0 = 0
                for g in range(nsegB):
                    seg = (cbsb - s0) // (nsegB - g)
                    nc.gpsimd.dma_gather(
                        gxB[:, s0 : s0 + seg, :], xhi[:],
                        s_idxB[:, (offB + s0) * 8 : (offB + s0 + seg) * 8],
                        seg * P, seg * P, D_IN, single_packet=False,
                        queue_num=qctr[0] % 4,
                    )
                    qctr[0] += 1
                    s0 += seg
                ohB = ohpB.tile([P, cbsb, P], f16, tag="ohB")
                nc.vector.tensor_tensor(
                    out=ohB[:],
                    in0=bcast_mid(s_dlocB[:, offB : offB + cbsb], P),
                    in1=rep_mid(s_iota[:], cbsb),
                    op=mybir.AluOpType.is_equal,
                )

            agg_ps = aggps.tile([P, nb * P], f32, space="PSUM", tag="aggps")
            a0 = 0
            b0 = 0
            for bl in range(nb):
                nchunks = ca[bl] + cb[bl]
                j = 0
                for c in range(ca[bl]):
                    nc.tensor.matmul(
                        agg_ps[:, bl * P : (bl + 1) * P],
                        lhsT=gxA[:, a0 + c, :],
                        rhs=ohA[:, a0 + c, :],
                        start=(j == 0),
                        stop=(j == nchunks - 1),
                    )
                    j += 1
                for c in range(cb[bl]):
                    nc.tensor.matmul(
                        agg_ps[:, bl * P : (bl + 1) * P],
                        lhsT=gxB[:, b0 + c, :],
                        rhs=ohB[:, b0 + c, :],
                        start=(j == 0),
                        stop=(j == nchunks - 1),
                    )
                    j += 1
                a0 += ca[bl]
                b0 += cb[bl]

            agg_s = aggsb.tile([P, nb * P], f16, tag="aggsb")
            nc.scalar.copy(out=agg_s[:], in_=agg_ps[:])

            proj_ps = projps.tile([D_OUT, nb * P], f32, space="PSUM", tag="projps")
            nc.tensor.matmul(proj_ps[:], lhsT=s_wt[:], rhs=agg_s[:],
                             start=True, stop=True)

            out_s = outsb.tile([D_OUT, nb * P], f32, tag="outsb")
            colsl = slice(sb * SB * P, sb * SB * P + nb * P)
            nc.vector.tensor_tensor(out=out_s[:], in0=proj_ps[:],
                                    in1=s_scale[:, colsl], op=mybir.AluOpType.mult)
            if has_bias:
                nc.vector.tensor_tensor(out=out_s[:], in0=out_s[:],
                                        in1=s_bias[:, colsl], op=mybir.AluOpType.add)
            nc.sync.dma_start(out=outT[:, colsl], in_=out_s[:])

            offA += casb
            offB += cbsb

    nc.compile()
    return nc


def _wrap_idx(idx_list):
    """[n] int16 -> [128, n//16] wrapped + replicated layout."""
    n = idx_list.shape[0]
    w16 = idx_list.reshape(n // 16, 16).T  # [16, n/16]
    return np.tile(w16, (8, 1)).astype(np.int16)


def kernel(x, W, b, row, col):
    global last_results
    x = np.asarray(x, dtype=np.float32)
    W = np.asarray(W, dtype=np.float32)
    b = np.asarray(b, dtype=np.float32)
    row = np.asarray(row).astype(np.int64)
    col = np.asarray(col).astype(np.int64)

    deg = np.bincount(row, minlength=N_NODES)
    recip = np.where(deg > 0, 1.0 / np.maximum(deg, 1), 0.0).astype(np.float32)
    mask = (deg > 0).astype(np.float32)

    # sort edges by (core, block, half)
    core = row // NPC
    local = row - core * NPC
    blk = local // P
    dloc = (local - blk * P).astype(np.int16)
    half = (col >= HALF).astype(np.int64)
    key = (core * NBLK + blk) * 2 + half
    order = np.argsort(key, kind="stable")
    ks = key[order]
    cs = col[order]
    dl = dloc[order]

    counts = np.bincount(ks, minlength=N_CORES * NBLK * 2).reshape(N_CORES, NBLK, 2)
    chunks = -(-counts // P)  # ceil
    CA = np.maximum(chunks[:, :, 0].max(axis=0), 1)  # [NBLK]
    CB = chunks[:, :, 1].max(axis=0)                 # [NBLK]
    CAtot = int(CA.sum())
    CBtot = int(CB.sum())

    starts = np.zeros(N_CORES * NBLK * 2 + 1, np.int64)
    np.cumsum(counts.reshape(-1), out=starts[1:])

    # per-core padded streams
    idxA_dev = np.zeros((N_CORES, P, CAtot * 8), np.int16)
    idxB_dev = np.zeros((N_CORES, P, max(CBtot, 1) * 8), np.int16)
    dlocA_dev = np.zeros((N_CORES, P, CAtot), np.float16)
    dlocB_dev = np.zeros((N_CORES, P, max(CBtot, 1)), np.float16)
    scale_dev = np.zeros((N_CORES, D_OUT, NPAD), np.float16)
    has_bias = bool(np.any(b))
    bias_dev = np.zeros((N_CORES, D_OUT, NPAD), np.float32) if has_bias else None

    for k in range(N_CORES):
        for h, (Cb, idx_dev, dloc_dev, base_sub) in enumerate(
            ((CA, idxA_dev, dlocA_dev, 0), (CB, idxB_dev, dlocB_dev, HALF))
        ):
            idx_stream = np.zeros(int(Cb.sum()) * P, np.int16)
            dl_stream = np.full(int(Cb.sum()) * P, -1.0, np.float16)
            off = 0
            for bidx in range(NBLK):
                g = (k * NBLK + bidx) * 2 + h
                s, e = starts[g], starts[g + 1]
                n = e - s
                idx_stream[off : off + n] = (cs[s:e] - base_sub).astype(np.int16)
                dl_stream[off : off + n] = dl[s:e].astype(np.float16)
                off += int(Cb[bidx]) * P
            if Cb.sum() == 0:
                continue
            # wrap per superblock call
            woff = 0
            soff = 0
            for sb in range(NSB):
                blocks = range(sb * SB, min(sb * SB + SB, NBLK))
                csb = int(sum(Cb[bb] for bb in blocks))
                if csb == 0:
                    continue
                n = csb * P
                idx_dev[k][:, woff * 8 : woff * 8 + n // 16] = _wrap_idx(
                    idx_stream[soff : soff + n]
                )
                woff += csb
                soff += n
            dloc_dev[k] = dl_stream.reshape(-1, P).T
        base = k * NPC
        scale_dev[k][:, :NPC] = recip[base : base + NPC][None, :]
        if has_bias:
            bias_dev[k][:, :NPC] = b[:, None] * mask[None, base : base + NPC]

    xlo = np.ascontiguousarray(x[:HALF]).astype(np.float16)
    xhi = np.ascontiguousarray(x[HALF:]).astype(np.float16)
    iota_t = np.tile(np.arange(P, dtype=np.float16), (P, 1))
    wt = np.ascontiguousarray(W.T).astype(np.float16)

    in_maps = []
    for k in range(N_CORES):
        in_maps.append(
            dict(
                xlo=xlo, xhi=xhi,
                idxA=idxA_dev[k], idxB=idxB_dev[k],
                dlocA=dlocA_dev[k], dlocB=dlocB_dev[k],
                iota=iota_t, wt=wt,
                scale=scale_dev[k],
            )
        )
        if has_bias:
            in_maps[-1]["biasr"] = bias_dev[k]

    cache_key = (tuple(CA.tolist()), tuple(CB.tolist()), has_bias)
    if cache_key not in _prog_cache:
        _prog_cache[cache_key] = _build_program(CA, CB, has_bias)
    nc = _prog_cache[cache_key]

    res = run_bass_kernel_spmd(nc, in_maps, core_ids=list(range(N_CORES)))
    last_results = res

    out = np.empty((N_NODES, D_OUT), np.float32)
    for k in range(N_CORES):
        out[k * NPC : (k + 1) * NPC] = res.results[k]["outT"][:, :NPC].T
    return out



# revision 19
# speedup vs baseline: 1.4331x; 1.1964x over previous
"""GNN mean-aggregator (h = xW^T + b; out[i] = mean_{(i,j) in E} h[j]) on 8 trn2 cores.

Strategy (graph/data parallel over destination nodes):
  - Each core owns a contiguous range of 6250 destination nodes.
  - Host sorts edges by destination, groups them into 128-destination blocks,
    splits each block's edges by source-node half (int16 gather index limit),
    and pads each (block, half) group to whole 128-edge chunks, uniformly
    across cores (SPMD: one program, per-core data).
  - Device: dma_gather fetches fp16 x rows per edge (edge-major chunks),
    a one-hot matrix built with a single broadcast is_equal per gather maps
    edges to their local destination, and TensorE matmuls accumulate
    sum_{e} x[col_e] per destination block in PSUM (feature-major).
    A second small matmul applies W^T, then the result is scaled by 1/deg
    (and bias, masked for deg=0) and written out.
"""
import sys

sys.path.insert(0, "/opt/trn_rl_repo")

from contextlib import ExitStack

import numpy as np

from concourse import bass, bacc, mybir, tile
from concourse.bass_utils import run_bass_kernel_spmd

N_NODES = 50000
N_EDGES = 800000
D_IN = 128
D_OUT = 64
N_CORES = 8
NPC = N_NODES // N_CORES      # 6250 destination nodes per core
P = 128
NBLK = (NPC + P - 1) // P     # 49 blocks of 128 destinations
NPAD = NBLK * P               # 6272 padded destinations
HALF = 32768                  # int16 gather-index boundary
SB = 4                        # blocks per superblock (gather granularity)
NSB = (NBLK + SB - 1) // SB   # 13 superblocks

_prog_cache = {}
last_results = None  # test harness introspection


def _build_program(CA, CB):
    """CA/CB: per-block chunk counts (uniform across cores)."""
    CA = list(CA)
    CB = list(CB)
    CAtot = sum(CA)
    CBtot = sum(CB)

    nc = bacc.Bacc("TRN2", target_bir_lowering=False, debug=False,
                   num_swdge_queues=4, dynamic_dma_scratch_size=16384)
    f16 = mybir.dt.float16
    f32 = mybir.dt.float32
    i16 = mybir.dt.int16

    xlo = nc.declare_dram_parameter("xlo", [HALF, D_IN], f16, isOutput=False)
    xhi = nc.declare_dram_parameter("xhi", [N_NODES - HALF, D_IN], f16, isOutput=False)
    idxA = nc.declare_dram_parameter("idxA", [P, CAtot * 8], i16, isOutput=False)
    idxB = nc.declare_dram_parameter("idxB", [P, max(CBtot, 1) * 8], i16, isOutput=False)
    dlocA = nc.declare_dram_parameter("dlocA", [P, CAtot], f16, isOutput=False)
    dlocB = nc.declare_dram_parameter("dlocB", [P, max(CBtot, 1)], f16, isOutput=False)
    iota = nc.declare_dram_parameter("iota", [P, P], f16, isOutput=False)
    wt = nc.declare_dram_parameter("wt", [D_IN, D_OUT], f16, isOutput=False)
    scale = nc.declare_dram_parameter("scale", [D_OUT, NPAD], f32, isOutput=False)
    biasr = nc.declare_dram_parameter("biasr", [D_OUT, NPAD], f32, isOutput=False)
    outT = nc.declare_dram_parameter("outT", [D_OUT, NPAD], f32, isOutput=True)

    def bcast_mid(ap, reps):
        # [P, C] -> [P, C, reps] via zero-stride inner dim
        return bass.AP(tensor=ap.tensor, offset=ap.offset,
                       ap=[ap.ap[0], ap.ap[1], [0, reps]])

    def rep_mid(ap, reps):
        # [P, n] -> [P, reps, n] via zero-stride middle dim
        return bass.AP(tensor=ap.tensor, offset=ap.offset,
                       ap=[ap.ap[0], [0, reps], ap.ap[1]])

    with tile.TileContext(nc) as tc, ExitStack() as ctx:
        consts = ctx.enter_context(tc.tile_pool(name="consts", bufs=1))
        gxpA = ctx.enter_context(tc.tile_pool(name="gxA", bufs=3))
        gxpB = ctx.enter_context(tc.tile_pool(name="gxB", bufs=3))
        ohpA = ctx.enter_context(tc.tile_pool(name="ohA", bufs=3))
        ohpB = ctx.enter_context(tc.tile_pool(name="ohB", bufs=3))
        aggsb = ctx.enter_context(tc.tile_pool(name="aggsb", bufs=3))
        outsb = ctx.enter_context(tc.tile_pool(name="outsb", bufs=3))
        aggps = ctx.enter_context(tc.tile_pool(name="aggps", bufs=3, space="PSUM"))
        projps = ctx.enter_context(tc.tile_pool(name="projps", bufs=2, space="PSUM"))

        s_iota = consts.tile([P, P], f16)
        s_wt = consts.tile([D_IN, D_OUT], f16)
        s_idxA = consts.tile([P, CAtot * 8], i16)
        s_idxB = consts.tile([P, max(CBtot, 1) * 8], i16)
        s_dlocA = consts.tile([P, CAtot], f16)
        s_dlocB = consts.tile([P, max(CBtot, 1)], f16)
        s_scale = consts.tile([D_OUT, NPAD], f32)
        s_bias = consts.tile([D_OUT, NPAD], f32)
        nc.sync.dma_start(out=s_iota[:], in_=iota[:])
        nc.sync.dma_start(out=s_wt[:], in_=wt[:])
        nc.sync.dma_start(out=s_idxA[:], in_=idxA[:])
        nc.sync.dma_start(out=s_idxB[:], in_=idxB[:])
        nc.sync.dma_start(out=s_dlocA[:], in_=dlocA[:])
        nc.sync.dma_start(out=s_dlocB[:], in_=dlocB[:])
        nc.sync.dma_start(out=s_scale[:], in_=scale[:])
        nc.sync.dma_start(out=s_bias[:], in_=biasr[:])

        offA = 0
        offB = 0
        qctr = [0]
        for sb in range(NSB):
            blocks = list(range(sb * SB, min(sb * SB + SB, NBLK)))
            nb = len(blocks)
            ca = [CA[b] for b in blocks]
            cb = [CB[b] for b in blocks]
            casb = sum(ca)
            cbsb = sum(cb)

            gxA = gxpA.tile([P, casb, D_IN], f16, tag="gxA")
            nsegA = -(-casb // 16)
            s0 = 0
            for g in range(nsegA):
                seg = (casb - s0) // (nsegA - g)
                nc.gpsimd.dma_gather(
                    gxA[:, s0 : s0 + seg, :], xlo[:],
                    s_idxA[:, (offA + s0) * 8 : (offA + s0 + seg) * 8],
                    seg * P, seg * P, D_IN, single_packet=False,
                    queue_num=qctr[0] % 4,
                )
                qctr[0] += 1
                s0 += seg
            ohA = ohpA.tile([P, casb, P], f16, tag="ohA")
            nc.vector.tensor_tensor(
                out=ohA[:],
                in0=bcast_mid(s_dlocA[:, offA : offA + casb], P),
                in1=rep_mid(s_iota[:], casb),
                op=mybir.AluOpType.is_equal,
            )
            if cbsb > 0:
                gxB = gxpB.tile([P, cbsb, D_IN], f16, tag="gxB")
                nsegB = -(-cbsb // 16)
                s0 = 0
                for g in range(nsegB):
                    seg = (cbsb - s0) // (nsegB - g)
                    nc.gpsimd.dma_gather(
                        gxB[:, s0 : s0 + seg, :], xhi[:],
                        s_idxB[:, (offB + s0) * 8 : (offB + s0 + seg) * 8],
                        seg * P, seg * P, D_IN, single_packet=False,
                        queue_num=qctr[0] % 4,
                    )
                    qctr[0] += 1
                    s0 += seg
                ohB = ohpB.tile([P, cbsb, P], f16, tag="ohB")
                nc.vector.tensor_tensor(
                    out=ohB[:],
                    in0=bcast_mid(s_dlocB[:, offB : offB + cbsb], P),
                    in1=rep_mid(s_iota[:], cbsb),
                    op=mybir.AluOpType.is_equal,
                )

            agg_ps = aggps.tile([P, nb * P], f32, space="PSUM", tag="aggps")
            a0 = 0
            b0 = 0
            for bl in range(nb):
                nchunks = ca[bl] + cb[bl]
                j = 0
                for c in range(ca[bl]):
                    nc.tensor.matmul(
                        agg_ps[:, bl * P : (bl + 1) * P],
                        lhsT=gxA[:, a0 + c, :],
                        rhs=ohA[:, a0 + c, :],
                        start=(j == 0),
                        stop=(j == nchunks - 1),
                    )
                    j += 1
                for c in range(cb[bl]):
                    nc.tensor.matmul(
                        agg_ps[:, bl * P : (bl + 1) * P],
                        lhsT=gxB[:, b0 + c, :],
                        rhs=ohB[:, b0 + c, :],
                        start=(j == 0),
                        stop=(j == nchunks - 1),
                    )
                    j += 1
                a0 += ca[bl]
                b0 += cb[bl]

            agg_s = aggsb.tile([P, nb * P], f16, tag="aggsb")
            nc.scalar.copy(out=agg_s[:], in_=agg_ps[:])

            proj_ps = projps.tile([D_OUT, nb * P], f32, space="PSUM", tag="projps")
            nc.tensor.matmul(proj_ps[:], lhsT=s_wt[:], rhs=agg_s[:],
                             start=True, stop=True)

            out_s = outsb.tile([D_OUT, nb * P], f32, tag="outsb")
            colsl = slice(sb * SB * P, sb * SB * P + nb * P)
            nc.vector.tensor_tensor(out=out_s[:], in0=proj_ps[:],
                                    in1=s_scale[:, colsl], op=mybir.AluOpType.mult)
            nc.vector.tensor_tensor(out=out_s[:], in0=out_s[:],
                                    in1=s_bias[:, colsl], op=mybir.AluOpType.add)
            nc.sync.dma_start(out=outT[:, colsl], in_=out_s[:])

            offA += casb
            offB += cbsb

    nc.compile()
    return nc


def _wrap_idx(idx_list):
    """[n] int16 -> [128, n//16] wrapped + replicated layout."""
    n = idx_list.shape[0]
    w16 = idx_list.reshape(n // 16, 16).T  # [16, n/16]
    return np.tile(w16, (8, 1)).astype(np.int16)


def kernel(x, W, b, row, col):
    global last_results
    x = np.asarray(x, dtype=np.float32)
    W = np.asarray(W, dtype=np.float32)
    b = np.asarray(b, dtype=np.float32)
    row = np.asarray(row).astype(np.int64)
    col = np.asarray(col).astype(np.int64)

    deg = np.bincount(row, minlength=N_NODES)
    recip = np.where(deg > 0, 1.0 / np.maximum(deg, 1), 0.0).astype(np.float32)
    mask = (deg > 0).astype(np.float32)

    # sort edges by (core, block, half)
    core = row // NPC
    local = row - core * NPC
    blk = local // P
    dloc = (local - blk * P).astype(np.int16)
    half = (col >= HALF).astype(np.int64)
    key = (core * NBLK + blk) * 2 + half
    order = np.argsort(key, kind="stable")
    ks = key[order]
    cs = col[order]
    dl = dloc[order]

    counts = np.bincount(ks, minlength=N_CORES * NBLK * 2).reshape(N_CORES, NBLK, 2)
    chunks = -(-counts // P)  # ceil
    CA = np.maximum(chunks[:, :, 0].max(axis=0), 1)  # [NBLK]
    CB = chunks[:, :, 1].max(axis=0)                 # [NBLK]
    CAtot = int(CA.sum())
    CBtot = int(CB.sum())

    starts = np.zeros(N_CORES * NBLK * 2 + 1, np.int64)
    np.cumsum(counts.reshape(-1), out=starts[1:])

    # per-core padded streams
    idxA_dev = np.zeros((N_CORES, P, CAtot * 8), np.int16)
    idxB_dev = np.zeros((N_CORES, P, max(CBtot, 1) * 8), np.int16)
    dlocA_dev = np.zeros((N_CORES, P, CAtot), np.float16)
    dlocB_dev = np.zeros((N_CORES, P, max(CBtot, 1)), np.float16)
    scale_dev = np.zeros((N_CORES, D_OUT, NPAD), np.float32)
    bias_dev = np.zeros((N_CORES, D_OUT, NPAD), np.float32)

    for k in range(N_CORES):
        for h, (Cb, idx_dev, dloc_dev, base_sub) in enumerate(
            ((CA, idxA_dev, dlocA_dev, 0), (CB, idxB_dev, dlocB_dev, HALF))
        ):
            idx_stream = np.zeros(int(Cb.sum()) * P, np.int16)
            dl_stream = np.full(int(Cb.sum()) * P, -1.0, np.float16)
            off = 0
            for bidx in range(NBLK):
                g = (k * NBLK + bidx) * 2 + h
                s, e = starts[g], starts[g + 1]
                n = e - s
                idx_stream[off : off + n] = (cs[s:e] - base_sub).astype(np.int16)
                dl_stream[off : off + n] = dl[s:e].astype(np.float16)
                off += int(Cb[bidx]) * P
            if Cb.sum() == 0:
                continue
            # wrap per superblock call
            woff = 0
            soff = 0
            for sb in range(NSB):
                blocks = range(sb * SB, min(sb * SB + SB, NBLK))
                csb = int(sum(Cb[bb] for bb in blocks))
                if csb == 0:
                    continue
                n = csb * P
                idx_dev[k][:, woff * 8 : woff * 8 + n // 16] = _wrap_idx(
                    idx_stream[soff : soff + n]
                )
                woff += csb
                soff += n
            dloc_dev[k] = dl_stream.reshape(-1, P).T
        base = k * NPC
        scale_dev[k][:, :NPC] = recip[base : base + NPC][None, :]
        bias_dev[k][:, :NPC] = b[:, None] * mask[None, base : base + NPC]

    xlo = np.ascontiguousarray(x[:HALF]).astype(np.float16)
    xhi = np.ascontiguousarray(x[HALF:]).astype(np.float16)
    iota_t = np.tile(np.arange(P, dtype=np.float16), (P, 1))
    wt = np.ascontiguousarray(W.T).astype(np.float16)

    in_maps = []
    for k in range(N_CORES):
        in_maps.append(
            dict(
                xlo=xlo, xhi=xhi,
                idxA=idxA_dev[k], idxB=idxB_dev[k],
                dlocA=dlocA_dev[k], dlocB=dlocB_dev[k],
                iota=iota_t, wt=wt,
                scale=scale_dev[k], biasr=bias_dev[k],
            )
        )

    cache_key = (tuple(CA.tolist()), tuple(CB.tolist()))
    if cache_key not in _prog_cache:
        _prog_cache[cache_key] = _build_program(CA, CB)
    nc = _prog_cache[cache_key]

    res = run_bass_kernel_spmd(nc, in_maps, core_ids=list(range(N_CORES)))
    last_results = res

    out = np.empty((N_NODES, D_OUT), np.float32)
    for k in range(N_CORES):
        out[k * NPC : (k + 1) * NPC] = res.results[k]["outT"][:, :NPC].T
    return out

